# revision 7
# baseline (speedup 1.0000x reference)
"""GAT kernel for Trainium2, SPMD over 8 NeuronCores.

Math: the reference GAT variant computes attention logits e[b,h,i,j] that do
NOT depend on j (the "untransposed Wh2" formulation), so softmax over a row
whose support (adj!=0) carries a constant value collapses to 1/deg(i) on the
support and 0 elsewhere (NEG_INF -> exp underflow -> exactly 0 in fp32).
Hence, per batch element b:

    out[b] = elu( diag(1/deg_b) @ (adj_b * adj_weight_b) @ (h_b @ W) )

with deg_b[i] = sum_j adj_b[i,j].  The result is head-independent and `a` is
unused.  Sharding: data-parallel over batch (B == n_cores == 8).

Device layouts are chosen host-side so the kernel needs no on-chip
transposes: h, adj, adj_weight are fed transposed (contraction index on the
SBUF partition dim), W is fed natural.  adj is fed as uint8 (values 0/1).

ELU identity used on device: elu(x) = min(exp(x) - 1, relu(x)), exact for
all x (including exp overflow -> inf, where min picks relu(x) = x).
"""

import numpy as np

import concourse.bass as bass
import concourse.tile as tile
from concourse import bacc, mybir
from concourse.bass import ts
from concourse.bass_utils import run_bass_kernel_spmd

B, N, D = 8, 512, 1024
P = 128  # SBUF partitions
NB = N // P  # 4 row blocks
DB = D // P  # 8 contraction blocks
FH = D // 512  # 2 free-dim halves of 512

F32 = mybir.dt.float32
F32R = mybir.dt.float32r
U8 = mybir.dt.uint8
AF = mybir.ActivationFunctionType
ALU = mybir.AluOpType


def build_nc():
    nc = bacc.Bacc("TRN2", target_bir_lowering=False, debug=False, num_devices=B)

    hT = nc.dram_tensor("hT", [D, N], F32R, kind="ExternalInput").ap()
    W = nc.dram_tensor("W", [D, D], F32R, kind="ExternalInput").ap()
    adjT = nc.dram_tensor("adjT", [N, N], U8, kind="ExternalInput").ap()
    adjwT = nc.dram_tensor("adjwT", [N, N], F32, kind="ExternalInput").ap()
    out = nc.dram_tensor("out", [N, D], F32, kind="ExternalOutput").ap()
    out_r = out.rearrange("(n p) f -> p n f", p=P)

    with tile.TileContext(nc) as tc:
        with (
            tc.tile_pool(name="singles", bufs=1) as singles,
            tc.tile_pool(name="work", bufs=4) as work,
            tc.tile_pool(name="outp", bufs=4) as outp,
            tc.tile_pool(name="psum", bufs=8, space="PSUM") as psum,
        ):
            # ---- resident SBUF tensors --------------------------------
            hT_d = [singles.tile([P, N], F32R, name=f"hT{d}", tag=f"hT{d}") for d in range(DB)]
            W_d = [singles.tile([P, D], F32R, name=f"W{d}", tag=f"W{d}") for d in range(DB)]
            adjT_j = [singles.tile([P, N], U8, name=f"aT{j}", tag=f"aT{j}") for j in range(NB)]
            adjwT_j = [singles.tile([P, N], F32, name=f"awT{j}", tag=f"awT{j}") for j in range(NB)]
            adjTf = singles.tile([P, NB, N], F32)
            S = singles.tile([P, N], F32)  # sum_j adjTf (partial deg)
            MT_sb = singles.tile([P, NB, N], F32R)  # (adj * adj_weight)^T
            Wh_sb = singles.tile([P, NB, D], F32R)  # [p, j_blk, f]
            ones = singles.tile([P, 1], F32)
            r_sb = singles.tile([P, NB], F32)  # 1/deg, column layout

            # ---- input DMAs, ordered to match PE consumption ----------
            # adjT first (deg chain), then h/W chunk pairs for streamed
            # MM1, adjwT slotted before the last two pairs (needed only
            # by MT -> MM2).
            for j in range(NB):
                nc.sync.dma_start(adjT_j[j], adjT[ts(j, P), :])
            for d in range(6):
                nc.sync.dma_start(hT_d[d], hT[ts(d, P), :])
                nc.sync.dma_start(W_d[d], W[ts(d, P), :])
            for j in range(NB):
                nc.sync.dma_start(adjwT_j[j], adjwT[ts(j, P), :])
            for d in range(6, DB):
                nc.sync.dma_start(hT_d[d], hT[ts(d, P), :])
                nc.sync.dma_start(W_d[d], W[ts(d, P), :])

            nc.vector.memset(ones, 1.0)

            # ---- DVE: adj convert, deg pre-sum ------------------------
            for j in range(NB):
                nc.vector.tensor_copy(adjTf[:, j], adjT_j[j])
            t01 = work.tile([P, N], F32, tag="s01")
            nc.vector.tensor_add(t01, adjTf[:, 0], adjTf[:, 1])
            nc.vector.tensor_add(S, adjTf[:, 2], adjTf[:, 3])
            nc.vector.tensor_add(S, t01, S)

            # ---- PE: deg[i] column vectors via ones-matmul ------------
            deg_ps = psum.tile([P, NB], F32, tag="mm")
            for i in range(NB):
                nc.tensor.matmul(
                    deg_ps[:, i : i + 1], S[:, ts(i, P)], ones, start=True, stop=True
                )
            nc.vector.reciprocal(r_sb, deg_ps)

            # ---- DVE: M^T = float(adjT) * adjwT (f32r out) ------------
            for j in range(NB):
                nc.vector.tensor_mul(MT_sb[:, j], adjTf[:, j], adjwT_j[j])

            # ---- PE MM1: Wh = h @ W, d-outer so chunks stream ---------
            ps1 = [psum.tile([P, 512], F32, name=f"ps1_{k}", tag="mm") for k in range(NB * FH)]
            for d in range(DB):
                for i in range(NB):
                    for f in range(FH):
                        nc.tensor.matmul(
                            ps1[i * FH + f],
                            hT_d[d][:, ts(i, P)],
                            W_d[d][:, ts(f, 512)],
                            start=(d == 0),
                            stop=(d == DB - 1),
                        )
            for i in range(NB):
                for f in range(FH):
                    nc.scalar.copy(Wh_sb[:, i, ts(f, 512)], ps1[i * FH + f])

            # ---- PE MM2 + fused scale + ELU ---------------------------
            # x = r[i] * psum;  elu(x) = min(exp(x) - 1, relu(x))
            for f in range(FH):
                for i in range(NB):
                    ps2 = psum.tile([P, 512], F32, tag="mm")
                    for j in range(NB):
                        nc.tensor.matmul(
                            ps2,
                            MT_sb[:, j, ts(i, P)],
                            Wh_sb[:, j, ts(f, 512)],
                            start=(j == 0),
                            stop=(j == NB - 1),
                        )
                    r_i = r_sb[:, i : i + 1]
                    exp_t = work.tile([P, 512], F32, tag="exp")
                    nc.scalar.activation(exp_t, ps2, AF.Exp, scale=r_i)
                    relu_t = work.tile([P, 512], F32, tag="relu")
                    nc.vector.tensor_scalar(
                        relu_t, ps2, r_i, 0.0, op0=ALU.mult, op1=ALU.max
                    )
                    o_t = outp.tile([P, 512], F32)
                    nc.vector.scalar_tensor_tensor(
                        o_t, exp_t, -1.0, relu_t, op0=ALU.add, op1=ALU.min
                    )
                    nc.sync.dma_start(out_r[:, i, ts(f, 512)], o_t)

    nc.compile()
    return nc


_NC = None


def _get_nc():
    global _NC
    if _NC is None:
        _NC = build_nc()
    return _NC


def _in_maps(h, adj, adj_weight, W):
    h = np.ascontiguousarray(np.asarray(h, dtype=np.float32))
    adj = np.asarray(adj)
    adj_weight = np.ascontiguousarray(np.asarray(adj_weight, dtype=np.float32))
    Wf = np.ascontiguousarray(np.asarray(W, dtype=np.float32).reshape(D, D))
    hT = np.ascontiguousarray(h.transpose(0, 2, 1))
    adjT = np.ascontiguousarray(adj.transpose(0, 2, 1).astype(np.uint8))
    adjwT = np.ascontiguousarray(adj_weight.transpose(0, 2, 1))
    return [
        {"hT": hT[b], "W": Wf, "adjT": adjT[b], "adjwT": adjwT[b]} for b in range(B)
    ]


def _run(h, adj, adj_weight, W, a=None, trace=False, **trace_kw):
    nc = _get_nc()
    res = run_bass_kernel_spmd(
        nc, _in_maps(h, adj, adj_weight, W), core_ids=list(range(B)),
        trace=trace, **trace_kw,
    )
    out = np.stack([res.results[c]["out"] for c in range(B)], axis=0)
    return out.astype(np.float32), res


def kernel(h, adj, adj_weight, W, a=None, **_ignored):
    out, _ = _run(h, adj, adj_weight, W)
    return out


# revision 8
# speedup vs baseline: 1.2578x; 1.2578x over previous
"""GAT kernel for Trainium2, SPMD over 8 NeuronCores.

Math: the reference GAT variant computes attention logits e[b,h,i,j] that do
NOT depend on j (the "untransposed Wh2" formulation), so softmax over a row
whose support (adj!=0) carries a constant value collapses to 1/deg(i) on the
support and 0 elsewhere (NEG_INF -> exp underflow -> exactly 0 in fp32).
Hence, per batch element b:

    out[b] = elu( diag(1/deg_b) @ (adj_b * adj_weight_b) @ (h_b @ W) )

with deg_b[i] = sum_j adj_b[i,j].  The result is head-independent and `a` is
unused.  Sharding: data-parallel over batch (B == n_cores == 8).

Device layouts are chosen host-side so the kernel needs no on-chip
transposes: h, adj, adj_weight are fed transposed (contraction index on the
SBUF partition dim), W is fed natural.  adj is fed as uint8 (values 0/1).

ELU identity used on device: elu(x) = min(exp(x) - 1, relu(x)), exact for
all x (including exp overflow -> inf, where min picks relu(x) = x).
"""

import numpy as np

import concourse.bass as bass
import concourse.tile as tile
from concourse import bacc, mybir
from concourse.bass import ts
from concourse.bass_utils import run_bass_kernel_spmd

B, N, D = 8, 512, 1024
P = 128  # SBUF partitions
NB = N // P  # 4 row blocks
DB = D // P  # 8 contraction blocks
FH = D // 512  # 2 free-dim halves of 512

F32 = mybir.dt.float32
F32R = mybir.dt.float32r
U8 = mybir.dt.uint8
F16 = mybir.dt.float16
AF = mybir.ActivationFunctionType
ALU = mybir.AluOpType


def build_nc():
    nc = bacc.Bacc("TRN2", target_bir_lowering=False, debug=False, num_devices=B)

    hT = nc.dram_tensor("hT", [D, N], F16, kind="ExternalInput").ap()
    W = nc.dram_tensor("W", [D, D], F16, kind="ExternalInput").ap()
    adjT = nc.dram_tensor("adjT", [N, N], U8, kind="ExternalInput").ap()
    adjwT = nc.dram_tensor("adjwT", [N, N], F32, kind="ExternalInput").ap()
    out = nc.dram_tensor("out", [N, D], F32, kind="ExternalOutput").ap()
    out_r = out.rearrange("(n p) f -> p n f", p=P)

    with tile.TileContext(nc) as tc:
        with (
            tc.tile_pool(name="singles", bufs=1) as singles,
            tc.tile_pool(name="work", bufs=4) as work,
            tc.tile_pool(name="outp", bufs=4) as outp,
            tc.tile_pool(name="psum", bufs=8, space="PSUM") as psum,
        ):
            # ---- resident SBUF tensors --------------------------------
            hT_d = [singles.tile([P, N], F16, name=f"hT{d}", tag=f"hT{d}") for d in range(DB)]
            W_d = [singles.tile([P, D], F16, name=f"W{d}", tag=f"W{d}") for d in range(DB)]
            adjT_j = [singles.tile([P, N], U8, name=f"aT{j}", tag=f"aT{j}") for j in range(NB)]
            adjwT_j = [singles.tile([P, N], F32, name=f"awT{j}", tag=f"awT{j}") for j in range(NB)]
            adjTf = singles.tile([P, NB, N], F32)
            S = singles.tile([P, N], F32)  # sum_j adjTf (partial deg)
            MT_sb = singles.tile([P, NB, N], F32R)  # (adj * adj_weight)^T
            Wh_sb = singles.tile([P, NB, D], F32R)  # [p, j_blk, f]
            ones = singles.tile([P, 1], F32)
            r_sb = singles.tile([P, NB], F32)  # 1/deg, column layout

            # ---- input DMAs, ordered to match PE consumption ----------
            # adjT first (deg chain), then h/W chunk pairs for streamed
            # MM1, adjwT slotted before the last two pairs (needed only
            # by MT -> MM2).
            for d in range(2):
                nc.sync.dma_start(hT_d[d], hT[ts(d, P), :])
                nc.sync.dma_start(W_d[d], W[ts(d, P), :])
            for j in range(NB):
                nc.sync.dma_start(adjT_j[j], adjT[ts(j, P), :])
            for d in range(2, DB):
                nc.sync.dma_start(hT_d[d], hT[ts(d, P), :])
                nc.sync.dma_start(W_d[d], W[ts(d, P), :])
            for j in range(NB):
                nc.sync.dma_start(adjwT_j[j], adjwT[ts(j, P), :])

            nc.vector.memset(ones, 1.0)

            # ---- DVE: adj convert, deg pre-sum ------------------------
            for j in range(NB):
                nc.vector.tensor_copy(adjTf[:, j], adjT_j[j])
            t01 = work.tile([P, N], F32, tag="s01")
            nc.vector.tensor_add(t01, adjTf[:, 0], adjTf[:, 1])
            nc.vector.tensor_add(S, adjTf[:, 2], adjTf[:, 3])
            nc.vector.tensor_add(S, t01, S)

            # ---- DVE: M^T = float(adjT) * adjwT (f32r out) ------------
            for j in range(NB):
                nc.vector.tensor_mul(MT_sb[:, j], adjTf[:, j], adjwT_j[j])

            # ---- PE MM1: Wh = h @ W, d-outer so chunks stream ---------
            ps1 = [psum.tile([P, 512], F32, name=f"ps1_{k}", tag="mm") for k in range(NB * FH)]
            for d in range(DB):
                for i in range(NB):
                    for f in range(FH):
                        nc.tensor.matmul(
                            ps1[i * FH + f],
                            hT_d[d][:, ts(i, P)],
                            W_d[d][:, ts(f, 512)],
                            start=(d == 0),
                            stop=(d == DB - 1),
                        )
            # deg matmuls fill the PE gap while Wh evacuates (ACT/DVE split,
            # f0 half first so MM2(f0) can start sooner).
            deg_ps = psum.tile([P, NB], F32, tag="mm")
            for i in range(NB):
                nc.tensor.matmul(
                    deg_ps[:, i : i + 1], S[:, ts(i, P)], ones, start=True, stop=True
                )
            for f in range(FH):
                for i in range(NB):
                    dst = Wh_sb[:, i, ts(f, 512)]
                    if i % 2 == 0:
                        nc.scalar.copy(dst, ps1[i * FH + f])
                    else:
                        nc.vector.tensor_copy(dst, ps1[i * FH + f])
            nc.vector.reciprocal(r_sb, deg_ps)

            # ---- PE MM2 + fused scale + ELU ---------------------------
            # x = r[i] * psum;  elu(x) = min(exp(x) - 1, relu(x))
            for f in range(FH):
                for i in range(NB):
                    ps2 = psum.tile([P, 512], F32, tag="mm")
                    for j in range(NB):
                        nc.tensor.matmul(
                            ps2,
                            MT_sb[:, j, ts(i, P)],
                            Wh_sb[:, j, ts(f, 512)],
                            start=(j == 0),
                            stop=(j == NB - 1),
                        )
                    r_i = r_sb[:, i : i + 1]
                    exp_t = work.tile([P, 512], F32, tag="exp")
                    nc.scalar.activation(exp_t, ps2, AF.Exp, scale=r_i)
                    relu_t = work.tile([P, 512], F32, tag="relu")
                    nc.vector.tensor_scalar(
                        relu_t, ps2, r_i, 0.0, op0=ALU.mult, op1=ALU.max
                    )
                    o_t = outp.tile([P, 512], F32)
                    nc.vector.scalar_tensor_tensor(
                        o_t, exp_t, -1.0, relu_t, op0=ALU.add, op1=ALU.min
                    )
                    nc.gpsimd.dma_start(out_r[:, i, ts(f, 512)], o_t)

    nc.compile()
    return nc


_NC = None


def _get_nc():
    global _NC
    if _NC is None:
        _NC = build_nc()
    return _NC


def _in_maps(h, adj, adj_weight, W):
    h = np.ascontiguousarray(np.asarray(h, dtype=np.float32))
    adj = np.asarray(adj)
    adj_weight = np.ascontiguousarray(np.asarray(adj_weight, dtype=np.float32))
    Wf = np.ascontiguousarray(np.asarray(W, dtype=np.float32).reshape(D, D).astype(np.float16))
    hT = np.ascontiguousarray(h.transpose(0, 2, 1).astype(np.float16))
    adjT = np.ascontiguousarray(adj.transpose(0, 2, 1).astype(np.uint8))
    adjwT = np.ascontiguousarray(adj_weight.transpose(0, 2, 1))
    return [
        {"hT": hT[b], "W": Wf, "adjT": adjT[b], "adjwT": adjwT[b]} for b in range(B)
    ]


def _run(h, adj, adj_weight, W, a=None, trace=False, **trace_kw):
    nc = _get_nc()
    res = run_bass_kernel_spmd(
        nc, _in_maps(h, adj, adj_weight, W), core_ids=list(range(B)),
        trace=trace, **trace_kw,
    )
    out = np.stack([res.results[c]["out"] for c in range(B)], axis=0)
    return out.astype(np.float32), res


def kernel(h, adj, adj_weight, W, a=None, **_ignored):
    out, _ = _run(h, adj, adj_weight, W)
    return out
